# revision 23
# baseline (speedup 1.0000x reference)
"""Distributed causal attention (softmax over the QUERY axis) on 8 TRN2
NeuronCores, written in Bass/Tile.

Sharding: the reference normalizes softmax over the query axis (axis=1), so
each key-column's softmax is independent. We therefore shard the KEY axis:
core pair (2b, 2b+1) handles batch b, with even cores owning even 128-row
k-tiles and odd cores owning odd k-tiles. The interleaving makes the
causal-sparsity-aware SPMD instruction graph identical on all cores (the one
asymmetric bit - the diagonal mask - is per-core input data).

Both projection weights of the score path are merged on the host into
M = Wq^T Wk (x32 for fp8 range), and M is contracted into the K SIDE:
scores^T = (M k_c^T)^T q^T. Because each core owns its k rows, khT = M k_c^T
is computed entirely locally (half the FLOPs of a q-side projection), and the
score matmul contracts khT against RAW q - a host-packed input - so no
cross-core exchange, collective fence, or shared-HBM round-trip is needed at
all (the previous q-side variant lost ~10us to the CC-firmware fence chain).
Phase K' and the score matmuls run fully in fp8 DoubleRow (256-row
contractions; rel-err 1.66e-2 vs the 2e-2 gate, predicted exactly by a host
numpy simulation of the quantization chain); the exp applies the 1/(32*sqrt(H))
scale so all fp8 operands stay in e4m3's normal range. The v-projection and PV
stay bf16 (fp8 there adds ~3-4% error, over the gate). Inputs are host-packed
partition-major and loaded as large contiguous descriptors (HWDGE
descriptor-gen is ~600ns/instruction and ring bandwidth scales with
descriptor size), interleaved across the SP/ACT rings in exact consumption
order: M/k chunks (phase K'), full qT (phase D), v/Wv (phase B). The
per-k softmax reciprocal is computed in phase D and folded into vh as phase B
produces each chunk. The host sums the two partial PV outputs per batch pair.
"""

from contextlib import ExitStack

import numpy as np
import ml_dtypes

import concourse.bass as bass
from concourse import bacc
import concourse.tile as tile
import concourse.mybir as mybir
from concourse.bass_utils import run_bass_kernel_spmd
from concourse.tile import ScopedClock

BATCH = 4


def _fast_drain_and_barrier(self, tick_clock, wait_clock):
    """Tile kernel-tail with sem-only all-engine barriers (the default
    drain+butterfly pair costs ~8us); the explicit drains with the global
    clock waits already cover all tracked work. The final-value waits are
    split across the SP and ACT engines so the ~20-entry walk runs in
    parallel instead of serially on SP."""
    from bass_rust import VectorClock

    vals = list(tick_clock.global_clock)
    h1 = VectorClock([v if i % 2 == 0 else 0 for i, v in enumerate(vals)])
    h2 = VectorClock([v if i % 2 == 1 else 0 for i, v in enumerate(vals)])
    d1 = self.nc.sync.drain()
    wait_clock.add_sem_waits(d1.ins, ScopedClock({None: h1}))
    d2 = self.nc.scalar.drain()
    wait_clock.add_sem_waits(d2.ins, ScopedClock({None: h2}))
    self.nc.all_engine_barrier(sem_only=True)
    assert self.sems is not None
    popped = self.nc._tile_sem_poison_stack.pop()
    assert popped is self._sem_poison
    self.nc.clear_and_free_semaphores(list(self.sems.allocated().values()))
    self.nc.all_engine_barrier(sem_only=True)


tile.TileContext._drain_and_barrier = _fast_drain_and_barrier

P = 128
SEQ = 2048
E = 1024
H = 1024
KL = 1024          # k columns per core (16 tiles / 2 cores * 128)
NE = E // P        # 8
NH = H // P        # 8
NKS = KL // P      # 8 k slots per core
NQT = SEQ // P     # 16 q tiles
NB = 512           # matmul free-dim / psum bank
NEP = NE // 2      # 4 DoubleRow contraction pairs
MASK_NEG = -51200.0  # pre-exp-scale; exp applies 1/1024 -> effective -50
N_WARMUP = 14   # sized so the warmup hands off seamlessly into phase K'
                # (any PE idle gap resets the HAM un-throttle window)

BF16 = mybir.dt.bfloat16
FP8 = mybir.dt.float8e4
F32 = mybir.dt.float32
nbf16 = ml_dtypes.bfloat16
nfp8 = ml_dtypes.float8_e4m3


def slot_chunks(j):
    """(ext, [(off, width), ...]) q-chunks for score slot j (relative to 256j)."""
    ext = SEQ - 256 * j
    chunks = []
    off = 0
    if j % 2 == 1:
        chunks.append((0, 256))
        off = 256
    while off < ext:
        chunks.append((off, NB))
        off += NB
    return ext, chunks


def build_nc():
    nc = bacc.Bacc("TRN2", target_bir_lowering=False, debug=False, num_devices=8)
    # Inputs arrive pre-packed partition-major ep-interleaved for DoubleRow
    # ([128, NEP, 2, X]: partition p, (ep, ko, x) <-> logical row
    # (2*ep+ko)*128+p, col x) so each tensor loads with a few DMAs of large
    # contiguous descriptors.
    qT = nc.dram_tensor("qT", [P, NEP, 2, SEQ], FP8, kind="ExternalInput").ap()
    kT = nc.dram_tensor("kT", [P, NEP, 2, KL], FP8, kind="ExternalInput").ap()
    vT = nc.dram_tensor("vT", [P, NE * KL], BF16, kind="ExternalInput").ap()
    wqT = nc.dram_tensor("wqT", [P, NEP, 2, H], FP8, kind="ExternalInput").ap()
    wvT = nc.dram_tensor("wvT", [P, NE * H], BF16, kind="ExternalInput").ap()
    mask = nc.dram_tensor("mask", [P, 256], F32, kind="ExternalInput").ap()
    out = nc.dram_tensor("out", [SEQ, H], BF16, kind="ExternalOutput").ap()

    with tile.TileContext(nc) as tc, ExitStack() as ctx:
        wpool = ctx.enter_context(tc.tile_pool(name="w", bufs=1))
        ktv = ctx.enter_context(tc.tile_pool(name="ktv", bufs=1))
        qtp = ctx.enter_context(tc.tile_pool(name="qtp", bufs=1))
        khpool = ctx.enter_context(tc.tile_pool(name="kh", bufs=1))
        vhpool = ctx.enter_context(tc.tile_pool(name="vh", bufs=NKS))
        prpool = ctx.enter_context(tc.tile_pool(name="pr", bufs=1))
        smpool = ctx.enter_context(tc.tile_pool(name="sm", bufs=1))
        ostpool = ctx.enter_context(tc.tile_pool(name="ost", bufs=10))
        psum = ctx.enter_context(tc.tile_pool(name="ps", bufs=8, space="PSUM"))

        # ---- PE warmup: dummy matmuls with no input deps run during the
        # initial DMA wait, releasing the HAM clock throttle early ----
        wrm = smpool.tile([P, 256], BF16, tag="wrm", name="wrm")
        nc.vector.memset(wrm[:], 0.0)
        wps = psum.tile([P, 256], F32, tag="ps", name="wps")
        for i in range(N_WARMUP):
            nc.tensor.matmul(wps[:], lhsT=wrm[:, :P], rhs=wrm[:],
                             start=(i == 0), stop=(i == N_WARMUP - 1))

        # Alternate input DMAs between the SP and ACT HWDGE rings so
        # descriptor generation parallelizes.
        def in_dma(e, *args):
            return (nc.sync if e % 2 == 0 else nc.scalar).dma_start(*args)

        # ---- input DMAs in exact consumption order ----
        # Phase K' inputs first as single-ep chunks spread over THREE rings
        # (SP, ACT, gpsimd) — the cold uncore runs each ring at ~95GB/s, and
        # Q's tail is bound by the LAST M/k chunk's arrival, so a third ring
        # pulls that in by ~2.5us. Then full qT (phase D), the mask on
        # gpsimd, and v/Wv (phase B).
        wq_sb = wpool.tile([P, NEP, 2, H], FP8, tag="wq", name="wq_sb")
        kt_sb = ktv.tile([P, NEP, 2, KL], FP8, tag="kt", name="kt_sb")

        def wq_dma(ep, eng):
            eng.dma_start(wq_sb[:, ep:ep + 1, :, :], wqT[:, ep:ep + 1, :, :])

        def kt_dma(ep, eng):
            eng.dma_start(kt_sb[:, ep:ep + 1, :, :], kT[:, ep:ep + 1, :, :])

        wq_dma(0, nc.sync)
        kt_dma(0, nc.scalar)
        wq_dma(2, nc.gpsimd)
        kt_dma(1, nc.sync)
        wq_dma(1, nc.scalar)
        kt_dma(2, nc.gpsimd)
        wq_dma(3, nc.sync)
        kt_dma(3, nc.scalar)

        qt_sb = qtp.tile([P, NEP, 2, SEQ], FP8, tag="qt", name="qt_sb")
        for c in range(4):
            in_dma(c, qt_sb[:, c:c + 1, :, :], qT[:, c:c + 1, :, :])

        msk = smpool.tile([P, 256], F32, tag="msk", name="msk")
        nc.gpsimd.dma_start(msk[:], mask[:])

        wv_sb = wpool.tile([P, NE * H], BF16, tag="wv", name="wv_sb")
        vt_sb = ktv.tile([P, NE * KL], BF16, tag="vt", name="vt_sb")
        in_dma(0, vt_sb[:], vT[:])
        in_dma(1, wv_sb[:], wvT[:])

        # ---- phase K': khT[a, kl] = (M k_c^T), fp8 DoubleRow (256-row
        # contraction per matmul), computed entirely from this core's k rows.
        # Output packed [p, ep', ko', kl] (a = (2ep'+ko')*128+p) so phase D
        # can slice it as DoubleRow lhsT pairs directly. ----
        kh_sb = khpool.tile([P, NEP, 2, KL], FP8, tag="kh", name="kh_sb")
        for kb in range(KL // NB):
            pts = [psum.tile([P, NB], F32, tag="ps", name=f"pk{kb}_{at}")
                   for at in range(NH)]
            for ep in range(NEP):
                for at in range(NH):
                    nc.tensor.matmul(
                        pts[at][:],
                        lhsT=wq_sb[:, ep:ep + 1, :, at * P:(at + 1) * P]
                            .squeeze(1),
                        rhs=kt_sb[:, ep:ep + 1, :, kb * NB:(kb + 1) * NB]
                            .squeeze(1),
                        start=(ep == 0),
                        stop=(ep == NEP - 1),
                        perf_mode=mybir.MatmulPerfMode.DoubleRow,
                    )
            for at in range(NH):
                nc.vector.tensor_copy(
                    kh_sb[:, at // 2:at // 2 + 1, at % 2:at % 2 + 1,
                          kb * NB:(kb + 1) * NB].squeeze(1).squeeze(1),
                    pts[at][:])

        # ---- phase D: scoresT -> exp -> den -> rec (vh scaling happens in
        # phase B as each vh chunk is produced) ----
        pr_sb = []
        recs = []
        for j in range(NKS):
            ext, chunks = slot_chunks(j)
            q0 = 256 * j
            pr = prpool.tile([P, ext], BF16, tag=f"pr{j}", name=f"pr{j}")
            accs = smpool.tile([P, len(chunks)], F32, tag=f"acc{j}", name=f"acc{j}")
            for ci, (off, w) in enumerate(chunks):
                pt = psum.tile([P, NB], F32, tag="ps", name=f"sp{j}_{ci}")
                for ep in range(NEP):
                    nc.tensor.matmul(
                        pt[:, :w],
                        lhsT=kh_sb[:, ep:ep + 1, :, j * P:(j + 1) * P]
                            .squeeze(1),
                        rhs=qt_sb[:, ep:ep + 1, :, q0 + off:q0 + off + w]
                            .squeeze(1),
                        start=(ep == 0),
                        stop=(ep == NEP - 1),
                        perf_mode=mybir.MatmulPerfMode.DoubleRow,
                    )
                if off == 0:
                    nc.vector.tensor_add(pt[:, :256], pt[:, :256], msk[:])
                nc.scalar.activation(
                    pr[:, off:off + w],
                    pt[:, :w],
                    mybir.ActivationFunctionType.Exp,
                    scale=float(1.0 / (np.sqrt(H) * 32.0)),
                    accum_out=accs[:, ci:ci + 1],
                )
            den = smpool.tile([P, 1], F32, tag=f"den{j}", name=f"den{j}")
            nc.vector.tensor_reduce(
                den[:], accs[:], axis=mybir.AxisListType.X, op=mybir.AluOpType.add
            )
            rec = smpool.tile([P, 1], F32, tag=f"rec{j}", name=f"rec{j}")
            nc.vector.reciprocal(rec[:], den[:])
            pr_sb.append(pr)
            recs.append(rec)

        # ---- phase B: vh[kl, h] (e-outer, 8 concurrent banks); each chunk is
        # scaled by the softmax reciprocal (from phase D) as it is copied ----
        vh_sb = [vhpool.tile([P, H], BF16, tag="vh", name=f"vh{j}")
                 for j in range(NKS)]
        for hb in range(H // NB):
            pts = [psum.tile([P, NB], F32, tag="ps", name=f"pp_vh{j}_{hb}")
                   for j in range(NKS)]
            for e in range(NE):
                for j in range(NKS):
                    nc.tensor.matmul(
                        pts[j][:],
                        lhsT=vt_sb[:, e * KL + j * P:e * KL + (j + 1) * P],
                        rhs=wv_sb[:, e * H + hb * NB:e * H + (hb + 1) * NB],
                        start=(e == 0),
                        stop=(e == NE - 1),
                    )
            for j in range(NKS):
                sl = vh_sb[j][:, hb * NB:(hb + 1) * NB]
                nc.vector.tensor_copy(sl, pts[j][:])
                # scale on gpsimd (idle here): halves DVE's phase-B load so
                # phase E's first PSUM alloc isn't gated on a DVE backlog
                nc.gpsimd.tensor_scalar_mul(sl, sl, recs[j][:])

        # ---- phase E: PV + output ----
        # Interleave small-t (few accumulation MMs) and big-t (many) so the
        # DVE copy + out-DMA drain keeps pace with PSUM-bank production, and
        # END on a big tile: its long accumulation hides every earlier
        # copy/DMA, so only ONE unit's drain is exposed after the last
        # matmul (that unit is split across DVE+ACT and the SP+ACT rings).
        pv_order = []
        lo, hi = 0, NQT - 1
        while lo <= hi:
            pv_order.append(lo)
            if lo < hi:
                pv_order.append(hi)
            lo += 1
            hi -= 1
        for idx, t in enumerate(pv_order):
            jmax = t // 2
            for hb in range(H // NB):
                pt = psum.tile([P, NB], F32, tag="ps", name=f"pv{t}_{hb}")
                for j in range(jmax + 1):
                    off = t * P - 256 * j
                    nc.tensor.matmul(
                        pt[:],
                        lhsT=pr_sb[j][:, off:off + P],
                        rhs=vh_sb[j][:, hb * NB:(hb + 1) * NB],
                        start=(j == 0),
                        stop=(j == jmax),
                    )
                ot = ostpool.tile([P, NB], BF16, tag="ost", name=f"ot{t}_{hb}")
                if idx == len(pv_order) - 1 and hb == 1:
                    # the very last unit: two halves drained fully in
                    # parallel (DVE copy + SP ring / ACT copy + ACT ring)
                    h2 = NB // 2
                    nc.vector.tensor_copy(ot[:, :h2], pt[:, :h2])
                    nc.sync.dma_start(
                        out[t * P:(t + 1) * P, hb * NB:hb * NB + h2],
                        ot[:, :h2])
                    nc.scalar.copy(ot[:, h2:], pt[:, h2:])
                    nc.scalar.dma_start(
                        out[t * P:(t + 1) * P, hb * NB + h2:(hb + 1) * NB],
                        ot[:, h2:])
                    continue
                nc.vector.tensor_copy(ot[:], pt[:])
                eng = nc.gpsimd if idx + 1 <= NQT - 4 else (
                    nc.sync if hb == 0 else nc.scalar)
                eng.dma_start(
                    out[t * P:(t + 1) * P, hb * NB:(hb + 1) * NB], ot[:]
                )

    nc.compile()
    return nc


# ---------------- host-side prep ----------------

def core_k_tiles(parity):
    return list(range(parity, 16, 2))


def _pack(m):
    """[NE*128, X] -> [128, NE*X]: partition-major so DMA descriptors are
    large and contiguous (row e*128+p, col x) -> (p, e*X+x)."""
    r, x = m.shape
    return np.ascontiguousarray(
        m.reshape(NE, P, x).transpose(1, 0, 2).reshape(P, NE * x)
    )


def _pack_ep(m):
    """[NE*128, X] -> [128, NE/2, 2, X] ep-major DoubleRow pairing:
    (p, ep, eo, x) <-> row (2*ep+eo)*128+p, col x."""
    r, x = m.shape
    return np.ascontiguousarray(
        m.reshape(NE // 2, 2, P, x).transpose(2, 0, 1, 3)
    )


def make_in_maps(q, k, v, Wq, Wk, Wv):
    """q,k,v: [4, 2048, 1024] f32; W*: [1024, 1024] f32 -> 8 per-core in_maps."""
    # merge the two score-side weights: scores = q (Wq^T Wk) k^T; M is
    # contracted into the k side on-device (khT = M k_c^T), so ship M^T
    # (contraction dim c on partitions).
    M = (Wq.T.astype(np.float64) @ Wk.astype(np.float64) * 32.0).astype(np.float32)
    wqT = _pack_ep(np.ascontiguousarray(M.T).astype(nfp8))
    wvT = _pack(Wv.T.astype(nbf16))

    kk = np.arange(P)[:, None]
    qq = np.arange(P)[None, :]
    tri = np.where(qq >= kk, 0.0, MASK_NEG).astype(np.float32)
    mask_even = np.concatenate([tri, np.zeros((P, P), np.float32)], axis=1)
    mask_odd = np.concatenate([np.full((P, P), MASK_NEG, np.float32), tri], axis=1)

    in_maps = []
    for c in range(8):
        b, parity = c // 2, c % 2
        rows = np.concatenate(
            [np.arange(g * P, (g + 1) * P) for g in core_k_tiles(parity)]
        )
        in_maps.append({
            "qT": _pack_ep(q[b].T.astype(nfp8)),
            "kT": _pack_ep(np.ascontiguousarray(k[b][rows].T).astype(nfp8)),
            "vT": _pack(v[b][rows].T.astype(nbf16)),
            "wqT": wqT,
            "wvT": wvT,
            "mask": mask_even if parity == 0 else mask_odd,
        })
    return in_maps


def combine_outputs(outs):
    """outs: list of 8 [2048, 1024] partial arrays -> [4, 2048, 1024]."""
    res = np.empty((4, SEQ, H), np.float32)
    for b in range(4):
        res[b] = outs[2 * b].astype(np.float32) + outs[2 * b + 1].astype(np.float32)
    return res


_NC_CACHE = []


def kernel(q, k, v, Wq, Wk, Wv):
    """Full inputs in, full output out; 8-core TRN2 SPMD inside."""
    q = np.asarray(q, dtype=np.float32)
    k = np.asarray(k, dtype=np.float32)
    v = np.asarray(v, dtype=np.float32)
    Wq = np.asarray(Wq, dtype=np.float32)
    Wk = np.asarray(Wk, dtype=np.float32)
    Wv = np.asarray(Wv, dtype=np.float32)

    if not _NC_CACHE:
        _NC_CACHE.append(build_nc())
    nc = _NC_CACHE[0]

    in_maps = make_in_maps(q, k, v, Wq, Wk, Wv)
    res = run_bass_kernel_spmd(nc, in_maps, core_ids=list(range(8)))
    outs = [res.results[i]["out"] for i in range(8)]
    return combine_outputs(outs)
